# revision 14
# baseline (speedup 1.0000x reference)
"""Correlation cost volume kernel for Trainium2 (8 NeuronCores, data-parallel over batch).

cost[b, i, h, x] = mean_c left[b,c,h,x] * right[b,c,h,x-i],  i in [0,48), zero for x < i.

Per core (one batch element), all data bf16 (host converts; left pre-scaled by 1/C):
  For each group of HG=8 h rows:
    l_t [C, 8*320], r_t [C, 8*368] (47 left-pad zeros + data + 1 zero col) in SBUF.
    Per h row, 3 matmuls G[a, j] = sum_c l[c, X0+a] r_pad[c, X0+j] into a psum
    bank slot (A[128x176] B[128x176] C[64x112] packed in one 512-col fp32 bank).
    Engine eviction (DVE/Act alternating, 2h per op) -> rect SBUF bf16
    (per-h 464-col slots). One plain dump rect -> scr DRAM; three diagonal
    readbacks (DRAM flat stride 3713 = row+1) -> band SBUF [128, 8*144]
    (band[a, h, ci*48+k] = G[a, ci-chunk, a+k] = cost[i=47-k, x=X0+a]).
    PE transposes (bf16): AB merged [128,96]->[96,128], C [64,48]->[48,64]
    into psum bf16 [96, 384] per 2h; engine copy -> outg [96, 8*192];
    3 strided DMAs write the (k, h, x) output (disparity reversed; host flips).
"""
import numpy as np
import ml_dtypes

import concourse.bacc as bacc
import concourse.mybir as mybir
import concourse.tile as tile
from concourse.ap import AP
from concourse.bass_utils import run_bass_kernel_spmd

B, C, H, W = 8, 128, 96, 320
D = 48
HG = 8          # h rows per group
NG = H // HG    # 12 groups
RPAD = W + D    # 368: 47 left zeros, W data, 1 right zero
CHUNKS = [(0, 128, 0), (128, 128, 176), (256, 64, 352)]  # (X0, M, gcol)
SLOT = 336      # rect cols per h row (3 x 112; A/B stored as 2 64-part strips)
HW = H * W

SHEAR_MODE = "dram"  # kept for test.py compat
_cache = {}


def _build(_mode="dram"):
    nc = bacc.Bacc("TRN2", target_bir_lowering=False, debug=False, num_devices=8)
    left = nc.dram_tensor("left", [C, HW], mybir.dt.bfloat16, kind="ExternalInput").ap()
    right = nc.dram_tensor("right", [C, HW], mybir.dt.bfloat16, kind="ExternalInput").ap()
    ident_in = nc.dram_tensor("ident", [128, 128], mybir.dt.bfloat16, kind="ExternalInput").ap()
    out = nc.dram_tensor("out", [D, HW], mybir.dt.bfloat16, kind="ExternalOutput").ap()
    scr = [nc.dram_tensor(f"scr_{p}", [C, HG * SLOT], mybir.dt.bfloat16).ap() for p in range(4)]

    with tile.TileContext(nc) as tc:
        with (
            tc.tile_pool(name="io", bufs=4) as io_pool,
            tc.tile_pool(name="rectp", bufs=3) as rect_pool,
            tc.tile_pool(name="bandp", bufs=4) as band_pool,
            tc.tile_pool(name="outp", bufs=4) as outg_pool,
            tc.tile_pool(name="const", bufs=1) as const_pool,
            tc.tile_pool(name="gps", bufs=3, space="PSUM") as g_pool,
            tc.tile_pool(name="bts", bufs=2, space="PSUM") as bt_pool,
        ):
            ident = const_pool.tile([128, 128], mybir.dt.bfloat16)
            nc.sync.dma_start(out=ident[:, :], in_=ident_in[:, :])

            def load_group(g):
                """Issue input loads for group g; returns (l_t, r_t) tiles."""
                h0 = g * HG
                l_t = io_pool.tile([C, HG * W], mybir.dt.bfloat16, tag="lt")
                r_t = io_pool.tile([C, HG * RPAD], mybir.dt.bfloat16, tag="rt")
                rtp = r_t.ap[0][0]
                # zero pads: cols [0:47] and col 367 of each h row. Pool bufs
                # rotate with period=io bufs, and loads only write the data
                # cols, so pads stay zero after the first rotation.
                if g < 4:
                    nc.gpsimd.memset(
                        AP(r_t.tensor, r_t.offset, [[rtp, C], [RPAD, HG], [1, D - 1]]), 0.0)
                    nc.gpsimd.memset(
                        AP(r_t.tensor, r_t.offset + RPAD - 1, [[rtp, C], [RPAD, HG], [1, 1]]), 0.0)
                nc.sync.dma_start(out=l_t[:, :], in_=left[:, h0 * W : (h0 + HG) * W])
                r_dst = AP(r_t.tensor, r_t.offset + (D - 1), [[rtp, C], [RPAD, HG], [1, W]])
                nc.scalar.dma_start(out=r_dst, in_=right[:, h0 * W : (h0 + HG) * W])
                return l_t, r_t

            pending = load_group(0)
            for g in range(NG):
                h0 = g * HG
                l_t, r_t = pending

                rect = rect_pool.tile([C, HG * SLOT], mybir.dt.bfloat16, tag="rect")
                band = band_pool.tile([C, HG * 3 * D], mybir.dt.bfloat16, tag="band")
                outg = outg_pool.tile([96, HG * 192], mybir.dt.bfloat16, tag="outg")
                rp = rect.ap[0][0]
                bp = band.ap[0][0]

                for p in range(4):  # 2h units
                    gt = g_pool.tile([128, 1024], mybir.dt.float32, tag="g")
                    gp = gt.ap[0][0]
                    for e in range(2):
                        hl = 2 * p + e
                        for X0, M, gcol in CHUNKS:
                            NW = M + D
                            nc.tensor.matmul(
                                gt[:M, 512 * e + gcol : 512 * e + gcol + NW],
                                l_t[:, hl * W + X0 : hl * W + X0 + M],
                                r_t[:, hl * RPAD + X0 : hl * RPAD + X0 + NW],
                                start=True, stop=True,
                            )
                    eng = nc.vector if p % 2 == 0 else nc.scalar
                    ev = eng.tensor_copy if p % 2 == 0 else eng.copy
                    # A/B strips: rect112[64t+p', hslot, ci, c] = G[64t+p', ci, 64t+c]
                    for t in range(2):
                        ev(
                            AP(rect.tensor, rect.offset + 64 * t * rp + 2 * p * SLOT,
                               [[rp, 64], [SLOT, 2], [112, 2], [1, 112]]),
                            AP(gt.tensor, gt.offset + 64 * t * gp + 64 * t,
                               [[gp, 64], [512, 2], [176, 2], [1, 112]]),
                        )
                    ev(
                        AP(rect.tensor, rect.offset + 2 * p * SLOT + 224,
                           [[rp, 64], [SLOT, 2], [1, 112]]),
                        AP(gt.tensor, gt.offset + 352, [[gp, 64], [512, 2], [1, 112]]),
                    )

                # prefetch next group's inputs before this group's late DMAs
                if g + 1 < NG:
                    pending = load_group(g + 1)

                scr_g = scr[g % 4]
                nc.gpsimd.dma_start(out=scr_g[:, :], in_=rect[:, :])
                SROW = HG * SLOT
                rbs = []
                for ci in range(2):  # A,B: 2 strips of 64 partitions each
                    for t in range(2):
                        rbs.append((
                            AP(scr_g.tensor, scr_g.offset + 64 * t * SROW + ci * 112,
                               [[SROW + 1, 64], [SLOT, HG], [1, D]]),
                            AP(band.tensor, band.offset + 64 * t * bp + ci * D,
                               [[bp, 64], [3 * D, HG], [1, D]]),
                        ))
                rbs.append((
                    AP(scr_g.tensor, scr_g.offset + 224,
                       [[SROW + 1, 64], [SLOT, HG], [1, D]]),
                    AP(band.tensor, band.offset + 2 * D,
                       [[bp, 64], [3 * D, HG], [1, D]]),
                ))
                for (s_, d_), e_ in zip(rbs, [nc.gpsimd, nc.scalar, nc.gpsimd, nc.scalar, nc.gpsimd]):
                    e_.dma_start(out=d_, in_=s_)

                for p in range(4):
                    bt = bt_pool.tile([96, 384], mybir.dt.bfloat16, tag="bt")
                    for e in range(2):
                        hl = 2 * p + e
                        nc.tensor.transpose(
                            bt[0:96, 192 * e : 192 * e + 128],
                            band[:, hl * 3 * D : hl * 3 * D + 96],
                            ident[:, :],
                        )
                        nc.tensor.transpose(
                            bt[0:48, 192 * e + 128 : 192 * e + 192],
                            band[0:64, hl * 3 * D + 96 : hl * 3 * D + 144],
                            ident[0:64, 0:64],
                        )
                    ceng = nc.vector.tensor_copy if p % 2 else nc.scalar.copy
                    ceng(outg[:, p * 384 : (p + 1) * 384], bt[:, :])

                # out DMAs: rev volume rev[k] = cost[47-k]; host flips.
                ogp = outg.ap[0][0]
                for part, coff, xoff, MW in ((0, 0, 0, 128), (48, 0, 128, 128), (0, 128, 256, 64)):
                    src = AP(outg.tensor, outg.offset + part * ogp + coff,
                             [[ogp, D], [192, HG], [1, MW]])
                    dst = AP(out.tensor, out.offset + h0 * W + xoff,
                             [[HW, D], [W, HG], [1, MW]])
                    nc.sync.dma_start(out=dst, in_=src)
    nc.compile()
    return nc


def _get_nc(_mode="dram"):
    if _mode not in _cache:
        _cache[_mode] = _build(_mode)
    return _cache[_mode]


def kernel(left_feature, right_feature):
    import os
    left_feature = np.asarray(left_feature, dtype=np.float32)
    right_feature = np.asarray(right_feature, dtype=np.float32)
    b, c, h, w = left_feature.shape
    assert (b, c, h, w) == (B, C, H, W)
    nc = _get_nc()
    ident = np.eye(128, dtype=np.float32).astype(ml_dtypes.bfloat16)
    lf = (left_feature * (1.0 / C)).astype(ml_dtypes.bfloat16)
    rf = right_feature.astype(ml_dtypes.bfloat16)
    in_maps = []
    for i in range(B):
        in_maps.append({
            "left": np.ascontiguousarray(lf[i].reshape(C, HW)),
            "right": np.ascontiguousarray(rf[i].reshape(C, HW)),
            "ident": ident,
        })
    trace = bool(os.environ.get("KERNEL_TRACE"))
    res = run_bass_kernel_spmd(nc, in_maps, core_ids=list(range(B)), trace=trace)
    if trace:
        print("HW exec time:", res.exec_time_ns, "ns")
    outs = []
    for i in range(B):
        rev = res.results[i]["out"].astype(np.float32).reshape(D, H, W)
        outs.append(rev[::-1])  # device wrote k = 47 - i
    return np.stack(outs, axis=0)


if __name__ == "__main__":
    rng = np.random.default_rng(0)
    lf = rng.standard_normal((B, C, H, W), dtype=np.float32)
    rf = rng.standard_normal((B, C, H, W), dtype=np.float32)
    got = kernel(lf, rf)
    for (bb, i, hh, xx) in [(0, 0, 0, 0), (0, 5, 10, 100), (1, 47, 95, 319), (2, 47, 3, 10), (3, 20, 50, 10)]:
        want = float(np.dot(lf[bb, :, hh, xx], rf[bb, :, hh, xx - i]) / C) if xx >= i else 0.0
        print((bb, i, hh, xx), "got", got[bb, i, hh, xx], "want", want)


# revision 15
# speedup vs baseline: 1.1425x; 1.1425x over previous
"""Correlation cost volume kernel for Trainium2 (8 NeuronCores, data-parallel over batch).

cost[b, i, h, x] = mean_c left[b,c,h,x] * right[b,c,h,x-i],  i in [0,48), zero for x < i.

Per core (one batch element), all data bf16 (host converts; left pre-scaled by 1/C):
  For each group of HG=8 h rows:
    l_t [C, 8*320], r_t [C, 8*368] (47 left-pad zeros + data + 1 zero col) in SBUF.
    Per h row, 3 matmuls G[a, j] = sum_c l[c, X0+a] r_pad[c, X0+j] into a psum
    bank slot (A[128x176] B[128x176] C[64x112] packed in one 512-col fp32 bank).
    Engine eviction (DVE/Act alternating, 2h per op) -> rect SBUF bf16
    (per-h 464-col slots). One plain dump rect -> scr DRAM; three diagonal
    readbacks (DRAM flat stride 3713 = row+1) -> band SBUF [128, 8*144]
    (band[a, h, ci*48+k] = G[a, ci-chunk, a+k] = cost[i=47-k, x=X0+a]).
    PE transposes (bf16): AB merged [128,96]->[96,128], C [64,48]->[48,64]
    into psum bf16 [96, 384] per 2h; engine copy -> outg [96, 8*192];
    3 strided DMAs write the (k, h, x) output (disparity reversed; host flips).
"""
import numpy as np
import ml_dtypes

import concourse.bacc as bacc
import concourse.mybir as mybir
import concourse.tile as tile
from concourse.ap import AP
from concourse.bass_utils import run_bass_kernel_spmd

B, C, H, W = 8, 128, 96, 320
D = 48
HG = 8          # h rows per group
NG = H // HG    # 12 groups
RPAD = W + D    # 368: 47 left zeros, W data, 1 right zero
CHUNKS = [(0, 128, 0), (128, 128, 176), (256, 64, 352)]  # (X0, M, gcol)
SLOT = 464      # rect cols per h row (176+176+112)
HW = H * W

SHEAR_MODE = "dram"  # kept for test.py compat
_cache = {}


def _build(_mode="dram"):
    nc = bacc.Bacc("TRN2", target_bir_lowering=False, debug=False, num_devices=8)
    left = nc.dram_tensor("left", [C, HW], mybir.dt.bfloat16, kind="ExternalInput").ap()
    right = nc.dram_tensor("right", [C, HW], mybir.dt.bfloat16, kind="ExternalInput").ap()
    ident_in = nc.dram_tensor("ident", [128, 128], mybir.dt.bfloat16, kind="ExternalInput").ap()
    out = nc.dram_tensor("out", [D, HW], mybir.dt.bfloat16, kind="ExternalOutput").ap()
    scr = [nc.dram_tensor(f"scr_{p}", [C, HG * SLOT], mybir.dt.bfloat16).ap() for p in range(4)]

    with tile.TileContext(nc) as tc:
        with (
            tc.tile_pool(name="io", bufs=4) as io_pool,
            tc.tile_pool(name="rectp", bufs=3) as rect_pool,
            tc.tile_pool(name="bandp", bufs=4) as band_pool,
            tc.tile_pool(name="outp", bufs=4) as outg_pool,
            tc.tile_pool(name="const", bufs=1) as const_pool,
            tc.tile_pool(name="gps", bufs=3, space="PSUM") as g_pool,
            tc.tile_pool(name="bts", bufs=2, space="PSUM") as bt_pool,
        ):
            ident = const_pool.tile([128, 128], mybir.dt.bfloat16)
            nc.sync.dma_start(out=ident[:, :], in_=ident_in[:, :])

            def load_group(g):
                """Issue input loads for group g; returns (l_t, r_t) tiles."""
                h0 = g * HG
                l_t = io_pool.tile([C, HG * W], mybir.dt.bfloat16, tag="lt")
                r_t = io_pool.tile([C, HG * RPAD], mybir.dt.bfloat16, tag="rt")
                rtp = r_t.ap[0][0]
                # zero pads: cols [0:47] and col 367 of each h row. Pool bufs
                # rotate with period=io bufs, and loads only write the data
                # cols, so pads stay zero after the first rotation.
                if g < 4:
                    nc.gpsimd.memset(
                        AP(r_t.tensor, r_t.offset, [[rtp, C], [RPAD, HG], [1, D - 1]]), 0.0)
                    nc.gpsimd.memset(
                        AP(r_t.tensor, r_t.offset + RPAD - 1, [[rtp, C], [RPAD, HG], [1, 1]]), 0.0)
                nc.sync.dma_start(out=l_t[:, :], in_=left[:, h0 * W : (h0 + HG) * W])
                r_dst = AP(r_t.tensor, r_t.offset + (D - 1), [[rtp, C], [RPAD, HG], [1, W]])
                nc.scalar.dma_start(out=r_dst, in_=right[:, h0 * W : (h0 + HG) * W])
                return l_t, r_t

            pending = load_group(0)
            for g in range(NG):
                h0 = g * HG
                l_t, r_t = pending

                rect = rect_pool.tile([C, HG * SLOT], mybir.dt.bfloat16, tag="rect")
                band = band_pool.tile([C, HG * 3 * D], mybir.dt.bfloat16, tag="band")
                outg = outg_pool.tile([96, HG * 192], mybir.dt.bfloat16, tag="outg")
                rp = rect.ap[0][0]
                bp = band.ap[0][0]

                for p in range(4):  # 2h units
                    gt = g_pool.tile([128, 1024], mybir.dt.float32, tag="g")
                    gp = gt.ap[0][0]
                    for e in range(2):
                        hl = 2 * p + e
                        for X0, M, gcol in CHUNKS:
                            NW = M + D
                            nc.tensor.matmul(
                                gt[:M, 512 * e + gcol : 512 * e + gcol + NW],
                                l_t[:, hl * W + X0 : hl * W + X0 + M],
                                r_t[:, hl * RPAD + X0 : hl * RPAD + X0 + NW],
                                start=True, stop=True,
                            )
                    eng = nc.vector if p % 2 == 0 else nc.scalar
                    ev = eng.tensor_copy if p % 2 == 0 else eng.copy
                    ev(
                        AP(rect.tensor, rect.offset + 2 * p * SLOT,
                           [[rp, 128], [SLOT, 2], [1, 352]]),
                        AP(gt.tensor, gt.offset, [[gp, 128], [512, 2], [1, 352]]),
                    )
                    ev(
                        AP(rect.tensor, rect.offset + 2 * p * SLOT + 352,
                           [[rp, 64], [SLOT, 2], [1, 112]]),
                        AP(gt.tensor, gt.offset + 352, [[gp, 64], [512, 2], [1, 112]]),
                    )

                # prefetch next group's inputs before this group's late DMAs
                if g + 1 < NG:
                    pending = load_group(g + 1)

                scr_g = scr[g % 4]
                nc.gpsimd.dma_start(out=scr_g[:, :], in_=rect[:, :])
                for ci, (X0, M, gcol) in enumerate(CHUNKS):
                    src_ = AP(scr_g.tensor, scr_g.offset + gcol,
                              [[HG * SLOT + 1, M], [SLOT, HG], [1, D]])
                    dst_ = AP(band.tensor, band.offset + ci * D,
                              [[bp, M], [3 * D, HG], [1, D]])
                    rb_eng = nc.gpsimd if ci != 1 else nc.scalar
                    rb_eng.dma_start(out=dst_, in_=src_)

                for p in range(4):
                    bt = bt_pool.tile([96, 384], mybir.dt.bfloat16, tag="bt")
                    for e in range(2):
                        hl = 2 * p + e
                        nc.tensor.transpose(
                            bt[0:96, 192 * e : 192 * e + 128],
                            band[:, hl * 3 * D : hl * 3 * D + 96],
                            ident[:, :],
                        )
                        nc.tensor.transpose(
                            bt[0:48, 192 * e + 128 : 192 * e + 192],
                            band[0:64, hl * 3 * D + 96 : hl * 3 * D + 144],
                            ident[0:64, 0:64],
                        )
                    ceng = nc.vector.tensor_copy if p % 2 else nc.scalar.copy
                    ceng(outg[:, p * 384 : (p + 1) * 384], bt[:, :])

                # out DMAs: rev volume rev[k] = cost[47-k]; host flips.
                ogp = outg.ap[0][0]
                for part, coff, xoff, MW in ((0, 0, 0, 128), (48, 0, 128, 128), (0, 128, 256, 64)):
                    src = AP(outg.tensor, outg.offset + part * ogp + coff,
                             [[ogp, D], [192, HG], [1, MW]])
                    dst = AP(out.tensor, out.offset + h0 * W + xoff,
                             [[HW, D], [W, HG], [1, MW]])
                    nc.sync.dma_start(out=dst, in_=src)
    nc.compile()
    return nc


def _get_nc(_mode="dram"):
    if _mode not in _cache:
        _cache[_mode] = _build(_mode)
    return _cache[_mode]


def kernel(left_feature, right_feature):
    import os
    left_feature = np.asarray(left_feature, dtype=np.float32)
    right_feature = np.asarray(right_feature, dtype=np.float32)
    b, c, h, w = left_feature.shape
    assert (b, c, h, w) == (B, C, H, W)
    nc = _get_nc()
    ident = np.eye(128, dtype=np.float32).astype(ml_dtypes.bfloat16)
    lf = (left_feature * (1.0 / C)).astype(ml_dtypes.bfloat16)
    rf = right_feature.astype(ml_dtypes.bfloat16)
    in_maps = []
    for i in range(B):
        in_maps.append({
            "left": np.ascontiguousarray(lf[i].reshape(C, HW)),
            "right": np.ascontiguousarray(rf[i].reshape(C, HW)),
            "ident": ident,
        })
    trace = bool(os.environ.get("KERNEL_TRACE"))
    res = run_bass_kernel_spmd(nc, in_maps, core_ids=list(range(B)), trace=trace)
    if trace:
        print("HW exec time:", res.exec_time_ns, "ns")
    outs = []
    for i in range(B):
        rev = res.results[i]["out"].astype(np.float32).reshape(D, H, W)
        outs.append(rev[::-1])  # device wrote k = 47 - i
    return np.stack(outs, axis=0)


if __name__ == "__main__":
    rng = np.random.default_rng(0)
    lf = rng.standard_normal((B, C, H, W), dtype=np.float32)
    rf = rng.standard_normal((B, C, H, W), dtype=np.float32)
    got = kernel(lf, rf)
    for (bb, i, hh, xx) in [(0, 0, 0, 0), (0, 5, 10, 100), (1, 47, 95, 319), (2, 47, 3, 10), (3, 20, 50, 10)]:
        want = float(np.dot(lf[bb, :, hh, xx], rf[bb, :, hh, xx - i]) / C) if xx >= i else 0.0
        print((bb, i, hh, xx), "got", got[bb, i, hh, xx], "want", want)
